# revision 1
# baseline (speedup 1.0000x reference)
"""Trainium2 Bass kernel for CDRExtractor (segment_reduce).

Input : segmentation_mask (64, 3, 512, 512) fp32
Output: (64, 5) fp32 = [cdr, disc_mean, cup_mean, disc_mean, cup_mean]

Sharding: pure data parallel, 8 samples per core across 8 cores; each core
streams its 24 MiB shard once (DMA roofline ~70us/core at ~358 GB/s).

Per-core algorithm (t-space formulation; 16 tiles of 2 samples x 128 rows):
  T = [x1-x0 | x2-x0]                 (POOL TT subtract - only add/sub/mult
                                       are walrus-legal on the Pool engine)
  F = exp(T)  (f0 == 1 implicitly)    (ACT, bf16 out)
  sadd = f1+f2                        (POOL)
  L = ln(1 + sadd); r = exp(-L)       (ACT; +1 via free activation bias.
                                       ACT Reciprocal/Rsqrt are banned; exp
                                       and ln share one act-table set)
  p-sums  Sum_w f*r                   (mostly DVE fused scalar_tensor_
                                       tensor w/ accum_out; 1 per tile (0
                                       on two tiles) via ACT exp(t-L) with
                                       fused fp32 accum_out, tuned per-tile
                                       so ACT/DVE/POOL busy are equal)
  d-counts Sum_w [f > max(f_oth,1)]   (DVE STT is_gt w/ accum; max(f,1)
                                       clamps via POOL tensor_scalar_max;
                                       count>0.5 == row contains
                                       argmax==label, exact)
  tail: PE transpose + ones-matmul over the (128,32) accumulators,
        iota+penalty reduce-min/max for ymin/ymax per (sample,label),
        heights = relu(ymax-ymin), cdr = h_cup/(h_disc+1e-6), means /= H*W.

Engine busy per core (CoreSim cost model): ACT ~76us, DVE ~73us, DMA ~76us,
Pool ~77us - all four at the memory roofline; end-to-end sim/HW-model 95.0us.
STT/TensorReduce have no 2x uop (1 elem/cycle); Pool accepts TT add/sub/
mult and tensor_scalar but rejects TT max/is_gt/STT/reduce at codegen,
which is what fixes this split. Fill is
minimized by a warm-up activation (act-table load at t~0), per-channel
DMAs for tile 0, and deferring const DMAs to the tail. HW-verified
rel err vs reference: 4.5e-05.
"""

import numpy as np
from contextlib import ExitStack

B, C, H, W = 64, 3, 512, 512
NCORES = 8
SPC = B // NCORES      # samples per core = 8
PAIRS = SPC // 2       # sample pairs per core = 4
NB = H // 128          # 128-row blocks = 4
HW = float(H * W)

_CACHE = {}


def _build():
    import concourse.bass as bass
    import concourse.bacc as bacc
    import concourse.mybir as mybir
    from concourse.tile import TileContext

    # Offer only the act-table set containing BOTH exp and ln (ids kept
    # aligned with act_info.json) so the table never reloads mid-kernel.
    if not _CACHE.get("act_patch"):
        _orig_tables = bacc.get_activation_tables

        def _only_ln_exp(arch):
            t = _orig_tables(arch)
            keep = "natural_log_exp_and_others"
            return {k: (v if k == keep else set()) for k, v in t.items()}

        bacc.get_activation_tables = _only_ln_exp
        _CACHE["act_patch"] = True

    f32 = mybir.dt.float32
    bf16 = mybir.dt.bfloat16
    Alu = mybir.AluOpType
    AFT = mybir.ActivationFunctionType
    X_AX = mybir.AxisListType.X

    nc = bacc.Bacc()
    x = nc.dram_tensor("x", (SPC, C, H, W), f32, kind="ExternalInput")
    iota_in = nc.dram_tensor("iota", (32, 128), f32, kind="ExternalInput")
    ident_in = nc.dram_tensor("ident", (128, 128), f32, kind="ExternalInput")
    ones_in = nc.dram_tensor("ones", (128, 1), f32, kind="ExternalInput")
    out = nc.dram_tensor("out", (5, SPC), f32, kind="ExternalOutput")

    with TileContext(nc) as tc, ExitStack() as ctx:
        cpool = ctx.enter_context(tc.tile_pool(name="consts", bufs=1))
        apool = ctx.enter_context(tc.tile_pool(name="accs", bufs=1))
        mpool = ctx.enter_context(tc.tile_pool(name="main", bufs=4))
        ppool = ctx.enter_context(tc.tile_pool(name="ps", bufs=1, space="PSUM"))

        # dummy activation on a memset tile: forces the (one-time) act
        # table load to run at t~0 instead of behind the first X DMA
        warm = cpool.tile([1, 16], bf16, tag="warm")
        nc.vector.memset(warm[:, :], 0.0)
        nc.scalar.activation(warm[:, :], warm[:, :], AFT.Exp)

        iota = cpool.tile([32, 128], f32, tag="iota")
        ident = cpool.tile([128, 128], f32, tag="ident")
        ones = cpool.tile([128, 1], f32, tag="ones")

        # accumulators: col j = b*8 + s
        RS1 = apool.tile([128, 32], f32, tag="RS1")  # row-sums of p1 (cup)
        RS2 = apool.tile([128, 32], f32, tag="RS2")  # row-sums of p2 (disc)
        DM1 = apool.tile([128, 32], f32, tag="DM1")  # row-max argmax margin lbl1
        DM2 = apool.tile([128, 32], f32, tag="DM2")

        def stage_a(t, b):
            """DMA the (2 samples x 128 rows x 3ch) tile."""
            X = mpool.tile([128, 2 * C * W], f32, tag="X", name=f"X_{t}_{b}",
                           bufs=5)
            if (t, b) == (0, 0):
                # fill latency: small per-(sample,channel) DMAs
                for si in range(2):
                    for ci in range(C):
                        src = x[2 * t + si, ci, b * 128:(b + 1) * 128, :]
                        off = (si * C + ci) * W
                        nc.sync.dma_start(X[:, off:off + W], src)
                return X
            src = x[2 * t:2 * t + 2, :, b * 128:(b + 1) * 128, :]
            src = src.rearrange("s c h w -> h s c w")
            Xv = X.rearrange("p (s c w) -> p s c w", s=2, c=C)
            nc.sync.dma_start(Xv, src)
            return X

        def stage_b1(t, b, X):
            """t-space: T = [x1-x0 | x2-x0] (POOL), F = exp(T) (ACT),
            sadd = f1+f2 (POOL)."""
            Xv = X.rearrange("p (s c w) -> p s c w", s=2, c=C)
            T32 = mpool.tile([128, 2048], f32, tag="T32",
                             name=f"T32_{t}_{b}", bufs=4)
            Tv = T32.rearrange("p (s l w) -> p s l w", s=2, l=2)
            F = mpool.tile([128, 2048], bf16, tag="F", name=f"F_{t}_{b}",
                           bufs=4)
            Fv = F.rearrange("p (s l w) -> p s l w", s=2, l=2)
            sadd = mpool.tile([128, 1024], bf16, tag="sadd",
                              name=f"sadd_{t}_{b}", bufs=4)
            saddv = sadd.rearrange("p (s w) -> p s w", s=2)
            if (t, b) == (0, 0):
                # per-sample halves: engines start after 3 channel DMAs
                # instead of 6 (pipeline fill)
                for si in range(2):
                    for li in range(2):
                        nc.gpsimd.tensor_tensor(
                            Tv[:, si:si + 1, li, :],
                            Xv[:, si:si + 1, li + 1, :],
                            Xv[:, si:si + 1, 0, :], Alu.subtract)
                    h = slice(si * 1024, (si + 1) * 1024)
                    nc.scalar.activation(F[:, h], T32[:, h], AFT.Exp)
                    nc.gpsimd.tensor_tensor(
                        saddv[:, si:si + 1], Fv[:, si:si + 1, 0, :],
                        Fv[:, si:si + 1, 1, :], Alu.add)
                return T32, F, sadd
            for li in range(2):
                nc.gpsimd.tensor_tensor(
                    Tv[:, :, li, :], Xv[:, :, li + 1, :], Xv[:, :, 0, :],
                    Alu.subtract)
            nc.scalar.activation(F[:, :], T32[:, :], AFT.Exp)
            nc.gpsimd.tensor_tensor(saddv, Fv[:, :, 0, :], Fv[:, :, 1, :],
                                    Alu.add)
            return T32, F, sadd

        def stage_b2a(t, b, T32, F, sadd):
            """L = ln(1 + f1 + f2) and r = exp(-L) (ACT)."""
            lns = mpool.tile([128, 1024], f32, tag="lns",
                             name=f"lns_{t}_{b}", bufs=4)
            nc.scalar.activation(lns[:, :], sadd[:, :], AFT.Ln, bias=1.0)
            rb = mpool.tile([128, 1024], bf16, tag="rb",
                            name=f"rb_{t}_{b}", bufs=4)
            nc.scalar.activation(rb[:, :], lns[:, :], AFT.Exp, scale=-1.0)
            return lns, rb

        def stage_b2b(t, b, T32, F, sadd, lns, rb):
            """p-sums: 1 of 4 via ACT exp(t-L)-with-accum (2 of 4 on a few
            tiles to equalize ACT/DVE busy), rest via DVE STT (f*r);
            argmax counts via DVE STT vs max(f_other, 1)."""
            k2 = False
            # k0 tiles: all 4 p-sums via DVE STT (drops the ACT p-exp on a
            # couple of tiles so ACT dips under the POOL/DMA pace)
            k0 = (4 * t + b) in (5, 10)
            U = mpool.tile([128, 1024], f32, tag="U", name=f"U_{t}_{b}",
                           bufs=2)
            if not k0:
                nc.gpsimd.tensor_tensor(
                    U[:, 0:512], T32[:, 0:512], lns[:, 0:512], Alu.subtract)
            if k2:
                # (si=1, li=0): t-slice at 1024, lns-slice at 512
                nc.gpsimd.tensor_tensor(
                    U[:, 512:1024], T32[:, 1024:1536], lns[:, 512:1024],
                    Alu.subtract)

            # MM = [max(f2,1) | max(f1,1)] per sample (argmax test
            # [f_l > max(f_other, 1)]); cheap 4x tensor_scalar on DVE
            Fv = F.rearrange("p (s l w) -> p s l w", s=2, l=2)
            MM = mpool.tile([128, 2048], bf16, tag="MM", name=f"MM_{t}_{b}",
                            bufs=2)
            MMv = MM.rearrange("p (s l w) -> p s l w", s=2, l=2)
            nc.gpsimd.tensor_scalar_max(MMv[:, :, 0, :], Fv[:, :, 1, :], 1.0)
            nc.gpsimd.tensor_scalar_max(MMv[:, :, 1, :], Fv[:, :, 0, :], 1.0)

            pscr = mpool.tile([128, 2048], bf16, tag="pscr",
                              name=f"pscr_{t}_{b}", bufs=2)
            dscr = mpool.tile([128, 2048], bf16, tag="dscr",
                              name=f"dscr_{t}_{b}", bufs=2)
            for si in range(2):
                s_g = 2 * t + si
                col = b * 8 + s_g
                for li, RS, DM in ((0, RS1, DM1), (1, RS2, DM2)):
                    sl = slice((si * 2 + li) * 512, (si * 2 + li + 1) * 512)
                    if li == 0 and (si == 0 or k2) and not k0:
                        # p-sum via ACT exp with fused fp32 row-sum
                        usl = slice(si * 512, (si + 1) * 512)
                        nc.scalar.activation(
                            pscr[:, sl], U[:, usl], AFT.Exp,
                            accum_out=RS[:, col:col + 1])
                    else:
                        rsl = slice(si * 512, (si + 1) * 512)
                        nc.vector.scalar_tensor_tensor(
                            pscr[:, sl], F[:, sl], 0.0, rb[:, rsl],
                            Alu.add, Alu.mult, accum_out=RS[:, col:col + 1])
                    # argmax presence count (exact): [f_l > max(f_other,1)]
                    nc.vector.scalar_tensor_tensor(
                        dscr[:, sl], F[:, sl], 0.0, MM[:, sl],
                        Alu.add, Alu.is_gt, accum_out=DM[:, col:col + 1])

        # 3-stage software pipeline: ACT->POOL->ACT round trips mean tile
        # i's ln runs after tile i+1's exp, and its U/p/d stage after tile
        # i+2's exp, so no engine waits on a same-tile cross-engine dep.
        tiles = [(t, b) for t in range(PAIRS) for b in range(NB)]
        pend1 = None  # awaiting b2a (ln)
        pend2 = None  # awaiting b2b (U, p-exps, d-counts)
        for i, (t, b) in enumerate(tiles):
            X = stage_a(t, b)
            T32, F, sadd = stage_b1(t, b, X)
            if i == 0:
                # eager first tile: shortest path to getting DVE going
                lns1, rb1 = stage_b2a(t, b, T32, F, sadd)
                stage_b2b(t, b, T32, F, sadd, lns1, rb1)
                continue
            if pend2 is not None:
                stage_b2b(*pend2)
                pend2 = None
            if pend1 is not None:
                lns1, rb1 = stage_b2a(*pend1)
                pend2 = (*pend1, lns1, rb1)
                pend1 = None
            pend1 = (t, b, T32, F, sadd)
        lns1, rb1 = stage_b2a(*pend1)
        if pend2 is not None:
            stage_b2b(*pend2)
        stage_b2b(*pend1, lns1, rb1)

        # ---- tail ----
        # const loads for the tail (emitted late so they don't delay the
        # first X tile on the SP DMA queue)
        nc.sync.dma_start(iota[:, :], iota_in[:, :])
        nc.sync.dma_start(ident[:, :], ident_in[:, :])
        nc.sync.dma_start(ones[:, :], ones_in[:, :])
        O = cpool.tile([1, 40], f32, tag="O")
        S12 = ppool.tile([1, 64], f32, tag="S12")
        nc.tensor.matmul(S12[:, 0:32], ones[:, :], RS1[:, :], start=True, stop=True)
        nc.tensor.matmul(S12[:, 32:64], ones[:, :], RS2[:, :], start=True, stop=True)

        heights = []
        for li, DM in enumerate((DM1, DM2)):
            TD = ppool.tile([32, 128], f32, tag=f"TD{li}")
            nc.tensor.transpose(TD[:, :], DM[:, :], ident[:, :])
            TL = cpool.tile([32, 128], f32, tag=f"TL{li}")
            nc.vector.tensor_copy(TL[:, :], TD[:, :])
            pen = cpool.tile([32, 128], f32, tag=f"pen{li}")
            nc.vector.tensor_scalar(pen[:, :], TL[:, :], 0.5, 1e6,
                                    Alu.is_lt, Alu.mult)
            cmin = cpool.tile([32, 128], f32, tag=f"cmin{li}")
            nc.gpsimd.tensor_tensor(cmin[:, :], pen[:, :], iota[:, :], Alu.add)
            cmax = cpool.tile([32, 128], f32, tag=f"cmax{li}")
            nc.gpsimd.tensor_tensor(cmax[:, :], iota[:, :], pen[:, :],
                                    Alu.subtract)
            Y = cpool.tile([32, 2], f32, tag=f"Y{li}")
            nc.vector.tensor_reduce(Y[:, 0:1], cmin[:, :], X_AX, op=Alu.min)
            nc.vector.tensor_reduce(Y[:, 1:2], cmax[:, :], X_AX, op=Alu.max)
            YTmin = ppool.tile([1, 32], f32, tag=f"YTmin{li}")
            YTmax = ppool.tile([1, 32], f32, tag=f"YTmax{li}")
            nc.tensor.transpose(YTmin[:, :], Y[:, 0:1], ident[0:32, 0:32])
            nc.tensor.transpose(YTmax[:, :], Y[:, 1:2], ident[0:32, 0:32])
            ymin8 = cpool.tile([1, 8], f32, tag=f"ymin{li}")
            ymax8 = cpool.tile([1, 8], f32, tag=f"ymax{li}")
            nc.vector.tensor_reduce(
                ymin8[:, :], YTmin[0:1, :].rearrange("p (b s) -> p s b", b=4),
                X_AX, op=Alu.min)
            nc.vector.tensor_reduce(
                ymax8[:, :], YTmax[0:1, :].rearrange("p (b s) -> p s b", b=4),
                X_AX, op=Alu.max)
            hL = cpool.tile([1, 8], f32, tag=f"h{li}")
            nc.vector.tensor_tensor(hL[:, :], ymax8[:, :], ymin8[:, :],
                                    Alu.subtract)
            nc.vector.tensor_scalar_max(hL[:, :], hL[:, :], 0.0)
            heights.append(hL)

        h_cup, h_disc = heights
        den = cpool.tile([1, 8], f32, tag="den")
        nc.vector.tensor_scalar_add(den[:, :], h_disc[:, :], 1e-6)
        rec = cpool.tile([1, 8], f32, tag="rec")
        nc.vector.reciprocal(rec[:, :], den[:, :])
        nc.vector.tensor_tensor(O[:, 0:8], h_cup[:, :], rec[:, :], Alu.mult)

        ms1 = cpool.tile([1, 8], f32, tag="ms1")
        ms2 = cpool.tile([1, 8], f32, tag="ms2")
        nc.vector.tensor_reduce(
            ms1[:, :], S12[0:1, 0:32].rearrange("p (b s) -> p s b", b=4),
            X_AX, op=Alu.add)
        nc.vector.tensor_reduce(
            ms2[:, :], S12[0:1, 32:64].rearrange("p (b s) -> p s b", b=4),
            X_AX, op=Alu.add)
        sc = 1.0 / HW
        nc.vector.tensor_scalar_mul(O[:, 8:16], ms2[:, :], sc)
        nc.vector.tensor_scalar_mul(O[:, 16:24], ms1[:, :], sc)
        nc.vector.tensor_scalar_mul(O[:, 24:32], ms2[:, :], sc)
        nc.vector.tensor_scalar_mul(O[:, 32:40], ms1[:, :], sc)

        nc.sync.dma_start(out[:, :], O[:, :])

    nc.finalize()
    return nc


def _get_nc():
    if "nc" not in _CACHE:
        _CACHE["nc"] = _build()
    return _CACHE["nc"]


def _host_inputs():
    iota = (np.arange(128, dtype=np.float32)[None, :]
            + 128.0 * np.repeat(np.arange(4, dtype=np.float32), 8)[:, None])
    ident = np.eye(128, dtype=np.float32)
    ones = np.ones((128, 1), dtype=np.float32)
    return iota, ident, ones


def _run(seg_mask, trace=False):
    from concourse.bass_utils import run_bass_kernel_spmd

    x = np.ascontiguousarray(np.asarray(seg_mask, dtype=np.float32))
    assert x.shape == (B, C, H, W)
    iota, ident, ones = _host_inputs()
    in_maps = [
        {"x": x[SPC * c:SPC * (c + 1)], "iota": iota, "ident": ident,
         "ones": ones}
        for c in range(NCORES)
    ]
    nc = _get_nc()
    res = run_bass_kernel_spmd(nc, in_maps, core_ids=list(range(NCORES)),
                               trace=trace)
    outs = []
    for c in range(NCORES):
        o = np.asarray(res.results[c]["out"]).reshape(5, SPC).T
        outs.append(o)
    full = np.concatenate(outs, axis=0).astype(np.float32)
    return full, res


def kernel(segmentation_mask):
    full, _ = _run(segmentation_mask, trace=False)
    return full



# revision 18
# speedup vs baseline: 1.2235x; 1.2235x over previous
"""Trainium2 Bass kernel for CDRExtractor (segment_reduce).

Input : segmentation_mask (64, 3, 512, 512) fp32
Output: (64, 5) fp32 = [cdr, disc_mean, cup_mean, disc_mean, cup_mean]

Sharding: pure data parallel, 8 samples per core across 8 cores.

Design vs the 95us baseline: the input DMA (75.8us of per-queue DMA work
for the 24 MiB shard in the cost model) is split across the three
DMA-capable queues (SP / Activation / Pool) so it overlaps with compute
instead of gating it.  Per (sample, channel) plane loads land as
(128, 4, 512) tiles with partition = h%128, free = (h//128, w).

Per-sample math (bf16 after the first subtract):
  T = [x1-x0 | x2-x0]          (Pool TT sub, fp32 in -> bf16 out)
  F = exp(T)                   (ACT)
  sadd = f1+f2                 (DVE TT 2x or Pool)
  denominator r = 1/(1+sadd):
     'act' samples: L = ln(1+sadd); r = exp(-L) w/ fused row-sum  (ACT)
     'dve' samples: r = reciprocal(sadd+1) on DVE (+ row-sum at 4x)
  p1 = f1*r (DVE TT 2x) + row-sum via tensor_scalar accum (DVE 4x)
  p2 row-sums via identity sum(p0+p1+p2) = N  (tail)
  d_l presence counts per (sample, label, 128-row block), exact:
     'fused': one STT  count[max(t_other,0) < t_l]   (DVE 1x)
     'B': m=max(t_other,0) (DVE 4x); g=t_l-m (Pool); count[g>0] (DVE 4x)
  tail: ones-matmul over the (128,32) accumulators, iota+penalty
        reduce-min/max for ymin/ymax, cdr = h_cup/(h_disc+1e-6).

First/last samples run in 128-row chunks to shorten fill and drain; the
drain sample's chunk loads are interleaved into the tail of the SP queue
so its chunk chains overlap the other engines' backlog.
"""

import numpy as np
from contextlib import ExitStack

B, C, H, W = 64, 3, 512, 512
NCORES = 8
SPC = B // NCORES      # samples per core = 8
NB = H // 128          # 128-row blocks = 4
HW = float(H * W)

_CACHE = {}

# ---- knobs ----
CFG = dict(
    chunk=("s0", "s7"),            # which of s0/s7 run chunked
    s0_front_only=True,            # s0: chunk only loads+subs+exp
    denom={},                      # per-sample denominator eng, default 'act'
    d1={},                         # per-sample 'fused' (default) | 'B'
    d2={s: "B" for s in range(8)},  # per-sample 'fused' | 'B'
    sadd_eng="dve",                # 'dve' | 'pool'
    pool_planes=(3, 4, 5, 6),      # samples whose c0 plane loads on Pool
    act_chunks=2,                  # how many s0c2 chunk loads on ACT queue
    consts_q="sp_late",            # 'act' | 'pool' | 'sp_late'
    s7_interleave=True,            # spread s7 chunk loads through SP queue
    tail_tt_eng="dve",             # cmin/cmax engine: 'dve' | 'pool'
    load_pos="last",               # 'split' | 'last'
    d_stage="mid",                 # 'back' | 'mid'
    spread_early=(1,),             # samples loaded across 3 queues
    stage_order="fmb",             # 'fmb' | 'mfb' | 'mbf'
    weave_s7=True,                 # interleave s7 chunk stages
    weave_lag=6,                   # iteration of s7 chunk-0 front
    s7_il_start=4,                 # iteration of first s7 chunk load
    sr_eng="act",                  # sum(r) via 'act' accum | 'dve' ts
)


def _build():
    import concourse.bass as bass
    import concourse.bacc as bacc
    import concourse.mybir as mybir
    from concourse.tile import TileContext

    if not _CACHE.get("act_patch"):
        _orig_tables = bacc.get_activation_tables

        def _only_ln_exp(arch):
            t = _orig_tables(arch)
            keep = "natural_log_exp_and_others"
            return {k: (v if k == keep else set()) for k, v in t.items()}

        bacc.get_activation_tables = _only_ln_exp
        _CACHE["act_patch"] = True

    f32 = mybir.dt.float32
    bf16 = mybir.dt.bfloat16
    Alu = mybir.AluOpType
    AFT = mybir.ActivationFunctionType
    X_AX = mybir.AxisListType.X

    CHUNKED = set()
    if "s0" in CFG["chunk"]:
        CHUNKED.add(0)
    if "s7" in CFG["chunk"]:
        CHUNKED.add(SPC - 1)

    nc = bacc.Bacc()
    x = nc.dram_tensor("x", (SPC, C, H, W), f32, kind="ExternalInput")
    iota_in = nc.dram_tensor("iota", (32, 128), f32, kind="ExternalInput")
    ident_in = nc.dram_tensor("ident", (128, 128), f32, kind="ExternalInput")
    ones_in = nc.dram_tensor("ones", (128, 1), f32, kind="ExternalInput")
    out = nc.dram_tensor("out", (5, SPC), f32, kind="ExternalOutput")

    with TileContext(nc) as tc, ExitStack() as ctx:
        QENG = dict(sp=nc.sync, act=nc.scalar, pool=nc.gpsimd)
        cpool = ctx.enter_context(tc.tile_pool(name="consts", bufs=1))
        apool = ctx.enter_context(tc.tile_pool(name="accs", bufs=1))
        xpool = ctx.enter_context(tc.tile_pool(name="xin", bufs=3))
        tpool = ctx.enter_context(tc.tile_pool(name="tmain", bufs=4))
        fpool = ctx.enter_context(tc.tile_pool(name="fmain", bufs=3))
        mpool = ctx.enter_context(tc.tile_pool(name="mid", bufs=2))
        rpool = ctx.enter_context(tc.tile_pool(name="rr", bufs=3))
        ppool = ctx.enter_context(tc.tile_pool(name="ps", bufs=1,
                                               space="PSUM"))

        warm = cpool.tile([1, 16], bf16, tag="warm")
        nc.vector.memset(warm[:, :], 0.0)
        nc.scalar.activation(warm[:, :], warm[:, :], AFT.Exp)

        iota = cpool.tile([32, 128], f32, tag="iota")
        ident = cpool.tile([128, 128], f32, tag="ident")
        ones = cpool.tile([128, 1], f32, tag="ones")

        def emit_consts():
            cq = QENG["sp" if CFG["consts_q"] == "sp_late"
                      else CFG["consts_q"]]
            cq.dma_start(iota[:, :], iota_in[:, :])
            cq.dma_start(ident[:, :], ident_in[:, :])
            cq.dma_start(ones[:, :], ones_in[:, :])

        if CFG["consts_q"] != "sp_late":
            emit_consts()

        # accumulators: col j = s*4 + e  (e = h//128 block)
        RS1 = apool.tile([128, 32], f32, tag="RS1")
        RSr = apool.tile([128, 32], f32, tag="RSr")
        DM1 = apool.tile([128, 32], f32, tag="DM1")
        DM2 = apool.tile([128, 32], f32, tag="DM2")
        for acc_t in (RS1, RSr, DM1, DM2):
            nc.vector.memset(acc_t[:, :], 0.0)

        junkA = cpool.tile([128, 2048], bf16, tag="junkA")
        junkB = cpool.tile([128, 2048], bf16, tag="junkB")

        X = {}
        Tt = {}
        Ff = {}
        Rr = {}

        def eslice(base, e):
            return slice(base + e * 512, base + (e + 1) * 512)

        def load_plane(s, c, qname="sp"):
            Xt = xpool.tile([128, NB, 512], f32, tag=f"X{c}",
                            name=f"X_{s}_{c}")
            X[(s, c)] = Xt
            src = x[s, c].rearrange("(e p) w -> p e w", p=128)
            QENG[qname].dma_start(Xt, src)

        def load_chunk(s, c, e, q):
            if (s, c) not in X:
                X[(s, c)] = xpool.tile([128, NB, 512], f32, tag=f"X{c}",
                                       name=f"X_{s}_{c}")
            src = x[s, c, e * 128:(e + 1) * 128, :]
            QENG[q].dma_start(X[(s, c)][:, e, :], src)

        def stage_front(s, erange=None):
            """T = [x1-x0 | x2-x0] (POOL), F = exp(T) (ACT)."""
            if s not in Tt:
                Tt[s] = tpool.tile([128, 4096], bf16, tag="T", name=f"T_{s}")
                Ff[s] = fpool.tile([128, 4096], bf16, tag="F", name=f"F_{s}")
            T, F = Tt[s], Ff[s]
            x0 = X[(s, 0)].rearrange("p e w -> p (e w)")
            x1 = X[(s, 1)].rearrange("p e w -> p (e w)")
            x2 = X[(s, 2)].rearrange("p e w -> p (e w)")
            if erange is None:
                nc.gpsimd.tensor_tensor(T[:, 0:2048], x1, x0, Alu.subtract)
                nc.gpsimd.tensor_tensor(T[:, 2048:4096], x2, x0,
                                        Alu.subtract)
                nc.scalar.activation(F[:, :], T[:, :], AFT.Exp)
                return
            Tv = T.rearrange("p (l e w) -> p l e w", l=2, e=NB)
            Fv = F.rearrange("p (l e w) -> p l e w", l=2, e=NB)
            for e in erange:
                nc.gpsimd.tensor_tensor(
                    T[:, eslice(0, e)], x1[:, eslice(0, e)],
                    x0[:, eslice(0, e)], Alu.subtract)
                nc.gpsimd.tensor_tensor(
                    T[:, eslice(2048, e)], x2[:, eslice(0, e)],
                    x0[:, eslice(0, e)], Alu.subtract)
                nc.scalar.activation(Fv[:, :, e, :], Tv[:, :, e, :], AFT.Exp)

        def stage_mid(s, erange=None):
            """sadd = f1+f2, denominator r (+ row-sums of r)."""
            T, F = Tt[s], Ff[s]
            if s not in Rr:
                Rr[s] = rpool.tile([128, 2048], bf16, tag="r", name=f"r_{s}")
                Rr[(s, "sadd")] = mpool.tile([128, 2048], bf16, tag="sadd",
                                             name=f"sa_{s}")
                if CFG["denom"].get(s, "act") == "dve":
                    Rr[(s, "aux")] = mpool.tile([128, 2048], bf16, tag="s1p",
                                                name=f"s1p_{s}")
                else:
                    Rr[(s, "aux")] = mpool.tile([128, 2048], bf16, tag="L",
                                                name=f"L_{s}")
            r, sadd, aux = Rr[s], Rr[(s, "sadd")], Rr[(s, "aux")]
            se = CFG["sadd_eng"]
            se = se.get(s, "dve") if isinstance(se, dict) else se
            saddf = nc.vector if se == "dve" else nc.gpsimd
            for e in ([None] if erange is None else erange):
                sl = slice(0, 2048) if e is None else eslice(0, e)
                col = s * 4 + (0 if e is None else e)
                saddf.tensor_tensor(
                    sadd[:, sl], F[:, sl],
                    F[:, 2048 + sl.start:2048 + sl.stop], Alu.add)
                if CFG["denom"].get(s, "act") == "dve":
                    nc.vector.tensor_scalar_add(aux[:, sl], sadd[:, sl], 1.0)
                    with nc.allow_low_precision(reason="bf16 softmax denom"):
                        nc.vector.reciprocal(r[:, sl], aux[:, sl])
                    nc.vector.tensor_scalar(
                        junkA[:, sl], r[:, sl], 1.0, 0.0, Alu.mult, Alu.add,
                        accum_out=RSr[:, col:col + 1])
                else:
                    nc.scalar.activation(aux[:, sl], sadd[:, sl], AFT.Ln,
                                         bias=1.0)
                    if CFG["sr_eng"] == "act":
                        nc.scalar.activation(r[:, sl], aux[:, sl], AFT.Exp,
                                             scale=-1.0,
                                             accum_out=RSr[:, col:col + 1])
                    else:
                        nc.scalar.activation(r[:, sl], aux[:, sl], AFT.Exp,
                                             scale=-1.0)
                        nc.vector.tensor_scalar(
                            junkA[:, sl], r[:, sl], 1.0, 0.0, Alu.mult,
                            Alu.add, accum_out=RSr[:, col:col + 1])

        def stage_back(s, erange=None, parts=("p", "d")):
            """p1 row-sums + d1/d2 presence counts."""
            T, F, r = Tt[s], Ff[s], Rr[s]
            key = (s, "pscr")
            if key not in Rr:
                Rr[key] = mpool.tile([128, 2048], bf16, tag="pscr",
                                     name=f"p_{s}")
            pscr = Rr[key]
            need_m = {}
            for li in (1, 2):
                if CFG[f"d{li}"].get(s, "fused") == "B":
                    mk = (s, f"m{li}")
                    if mk not in Rr:
                        Rr[mk] = rpool.tile([128, 2048], bf16, tag=f"m{li}",
                                            name=f"m{li}_{s}")
                        Rr[(s, f"g{li}")] = mpool.tile(
                            [128, 2048], bf16, tag=f"g{li}", name=f"g{li}_{s}")
                    need_m[li] = (Rr[mk], Rr[(s, f"g{li}")])
            for e in ([None] if erange is None else erange):
                sl = slice(0, 2048) if e is None else eslice(0, e)
                col0 = s * 4 + (0 if e is None else e)
                if "p" in parts:
                    nc.vector.tensor_tensor(pscr[:, sl], F[:, sl], r[:, sl],
                                            Alu.mult)
                    nc.vector.tensor_scalar(
                        junkB[:, sl], pscr[:, sl], 1.0, 0.0, Alu.mult,
                        Alu.add, accum_out=RS1[:, col0:col0 + 1])
                if "d" not in parts:
                    continue
                # own-label slice vs other-label slice per label
                for li in (1, 2):
                    own = 0 if li == 1 else 2048
                    oth = 2048 - own
                    if li in need_m:
                        m, g = need_m[li]
                        nc.vector.tensor_scalar_max(
                            m[:, sl], T[:, oth + sl.start:oth + sl.stop], 0.0)
                        nc.gpsimd.tensor_tensor(
                            g[:, sl], T[:, own + sl.start:own + sl.stop],
                            m[:, sl], Alu.subtract)
                es = range(NB) if e is None else (e,)
                DMs = {1: DM1, 2: DM2}
                for ee in es:
                    col = s * 4 + ee
                    for li in (1, 2):
                        own = 0 if li == 1 else 2048
                        oth = 2048 - own
                        jnk = junkA if li == 1 else junkB
                        if li in need_m:
                            _, g = need_m[li]
                            nc.vector.tensor_scalar(
                                jnk[:, eslice(0, ee)], g[:, eslice(0, ee)],
                                0.0, 0.0, Alu.is_gt, Alu.add,
                                accum_out=DMs[li][:, col:col + 1])
                        else:
                            nc.vector.scalar_tensor_tensor(
                                jnk[:, eslice(0, ee)], T[:, eslice(oth, ee)],
                                0.0, T[:, eslice(own, ee)], Alu.max,
                                Alu.is_lt, accum_out=DMs[li][:, col:col + 1])

        # ---- software-pipelined emission ----
        s_last = SPC - 1
        head_chunked = 0 in CHUNKED
        tail_chunked = s_last in CHUNKED

        def emit_loads(i, part):
            # part 'sp': SP-queue loads; part 'eng': Pool/ACT-queue loads
            if i == 0:
                if head_chunked:
                    na = CFG["act_chunks"]
                    if part == "sp":
                        for e in range(NB):
                            load_chunk(0, 0, e, "sp")
                        for e in range(na, NB):
                            load_chunk(0, 2, e, "sp")
                    else:
                        for e in range(NB):
                            load_chunk(0, 1, e, "pool")
                        for e in range(na):
                            load_chunk(0, 2, e, "act")
                else:
                    if part == "sp":
                        for c in range(C):
                            load_plane(0, c, "sp")
                return
            if i >= SPC:
                return
            if i == s_last and tail_chunked:
                if part == "sp" and not CFG["s7_interleave"]:
                    for c in range(C):
                        for e in range(NB):
                            load_chunk(s_last, c, e, "sp")
                return
            if i in CFG["spread_early"]:
                qmap = {0: "pool", 1: "sp", 2: "act"}
                for c in range(C):
                    if (part == "eng") == (qmap[c] != "sp"):
                        load_plane(i, c, qmap[c])
                return
            on_pool = i in CFG["pool_planes"]
            for c in range(C):
                is_pool = on_pool and c == 0
                if (part == "eng") == is_pool:
                    load_plane(i, c, "pool" if is_pool else "sp")

        def emit_s7_interleaved(i):
            # spread s7's 12 chunk loads over iterations 4..7 (3 per iter)
            if not (tail_chunked and CFG["s7_interleave"]):
                return
            st = CFG["s7_il_start"]
            if st <= i <= st + 3:
                e = i - st
                for c in range(C):
                    load_chunk(s_last, c, e, "sp")

        def mb_chunked(j):
            if j not in CHUNKED:
                return None
            if j == 0 and CFG["s0_front_only"]:
                return None
            return range(NB)

        d_in_mid = CFG["d_stage"] == "mid"

        def skipw(j):
            return CFG["weave_s7"] and tail_chunked and j == s_last

        def do_front(i):
            j = i - 2
            if 0 <= j < SPC and not skipw(j):
                er = range(NB) if j in CHUNKED else None
                stage_front(j, er)

        def do_mid(i):
            j = i - 3
            if 0 <= j < SPC and not skipw(j):
                stage_mid(j, mb_chunked(j))
                if d_in_mid:
                    stage_back(j, mb_chunked(j), parts=("d",))

        def do_back(i):
            j = i - 4
            if 0 <= j < SPC and not skipw(j):
                stage_back(j, mb_chunked(j),
                           parts=("p",) if d_in_mid else ("p", "d"))

        SMAP = dict(f=do_front, m=do_mid, b=do_back)
        weave = CFG["weave_s7"] and tail_chunked

        wl = CFG["weave_lag"]

        def do_weave(i):
            if not weave:
                return
            e = i - wl
            if 0 <= e < NB:
                stage_front(s_last, (e,))
            e = i - wl - 1
            if 0 <= e < NB:
                stage_mid(s_last, (e,))
                if d_in_mid:
                    stage_back(s_last, (e,), parts=("d",))
            e = i - wl - 2
            if 0 <= e < NB:
                stage_back(s_last, (e,),
                           parts=("p",) if d_in_mid else ("p", "d"))

        for i in range(SPC + 4):
            if CFG["load_pos"] == "split":
                emit_loads(i, "sp")
                emit_s7_interleaved(i)
            ordered = [SMAP[ch] for ch in CFG["stage_order"]]
            ordered[0](i)
            do_weave(i)
            if CFG["load_pos"] == "split":
                emit_loads(i, "eng")
            for fn in ordered[1:]:
                fn(i)
            if CFG["load_pos"] == "last":
                emit_loads(i, "sp")
                emit_loads(i, "eng")
                emit_s7_interleaved(i)
            if i == SPC - 1 and CFG["consts_q"] == "sp_late":
                emit_consts()

        # ---- tail ----
        O = cpool.tile([1, 40], f32, tag="O")
        S1 = ppool.tile([1, 64], f32, tag="S1")
        nc.tensor.matmul(S1[:, 0:32], ones[:, :], RS1[:, :], start=True,
                         stop=True)
        nc.tensor.matmul(S1[:, 32:64], ones[:, :], RSr[:, :], start=True,
                         stop=True)

        heights = []
        for li, DM in enumerate((DM1, DM2)):
            TD = ppool.tile([32, 128], f32, tag=f"TD{li}")
            nc.tensor.transpose(TD[:, :], DM[:, :], ident[:, :])
            pen = cpool.tile([32, 128], f32, tag=f"pen{li}")
            nc.vector.tensor_scalar(pen[:, :], TD[:, :], 0.5, 1e6,
                                    Alu.is_lt, Alu.mult)
            teng = nc.vector if CFG["tail_tt_eng"] == "dve" else nc.gpsimd
            cmin = cpool.tile([32, 128], f32, tag=f"cmin{li}")
            teng.tensor_tensor(cmin[:, :], pen[:, :], iota[:, :], Alu.add)
            cmax = cpool.tile([32, 128], f32, tag=f"cmax{li}")
            teng.tensor_tensor(cmax[:, :], iota[:, :], pen[:, :],
                               Alu.subtract)
            Y = cpool.tile([32, 2], f32, tag=f"Y{li}")
            nc.vector.tensor_reduce(Y[:, 0:1], cmin[:, :], X_AX, op=Alu.min)
            nc.vector.tensor_reduce(Y[:, 1:2], cmax[:, :], X_AX, op=Alu.max)
            YTmin = ppool.tile([1, 32], f32, tag=f"YTmin{li}")
            YTmax = ppool.tile([1, 32], f32, tag=f"YTmax{li}")
            nc.tensor.transpose(YTmin[:, :], Y[:, 0:1], ident[0:32, 0:32])
            nc.tensor.transpose(YTmax[:, :], Y[:, 1:2], ident[0:32, 0:32])
            ymin8 = cpool.tile([1, 8], f32, tag=f"ymin{li}")
            ymax8 = cpool.tile([1, 8], f32, tag=f"ymax{li}")
            nc.vector.tensor_reduce(
                ymin8[:, :],
                YTmin[0:1, :].rearrange("p (s e) -> p s e", e=4),
                X_AX, op=Alu.min)
            nc.vector.tensor_reduce(
                ymax8[:, :],
                YTmax[0:1, :].rearrange("p (s e) -> p s e", e=4),
                X_AX, op=Alu.max)
            hL = cpool.tile([1, 8], f32, tag=f"h{li}")
            nc.vector.tensor_tensor(hL[:, :], ymax8[:, :], ymin8[:, :],
                                    Alu.subtract)
            nc.vector.tensor_scalar_max(hL[:, :], hL[:, :], 0.0)
            heights.append(hL)

        h_cup, h_disc = heights
        den = cpool.tile([1, 8], f32, tag="den")
        nc.vector.tensor_scalar_add(den[:, :], h_disc[:, :], 1e-6)
        rec = cpool.tile([1, 8], f32, tag="rec")
        nc.vector.reciprocal(rec[:, :], den[:, :])
        nc.vector.tensor_tensor(O[:, 0:8], h_cup[:, :], rec[:, :], Alu.mult)

        s1tot = cpool.tile([1, 8], f32, tag="s1tot")
        srtot = cpool.tile([1, 8], f32, tag="srtot")
        nc.vector.tensor_reduce(
            s1tot[:, :], S1[0:1, 0:32].rearrange("p (s e) -> p s e", e=4),
            X_AX, op=Alu.add)
        nc.vector.tensor_reduce(
            srtot[:, :], S1[0:1, 32:64].rearrange("p (s e) -> p s e", e=4),
            X_AX, op=Alu.add)
        p2a = cpool.tile([1, 8], f32, tag="p2a")
        nc.vector.tensor_scalar(p2a[:, :], srtot[:, :], -1.0, HW,
                                Alu.mult, Alu.add)
        p2tot = cpool.tile([1, 8], f32, tag="p2tot")
        nc.vector.tensor_tensor(p2tot[:, :], p2a[:, :], s1tot[:, :],
                                Alu.subtract)
        sc = 1.0 / HW
        nc.vector.tensor_scalar_mul(O[:, 8:16], p2tot[:, :], sc)
        nc.vector.tensor_scalar_mul(O[:, 16:24], s1tot[:, :], sc)
        nc.vector.tensor_scalar_mul(O[:, 24:32], p2tot[:, :], sc)
        nc.vector.tensor_scalar_mul(O[:, 32:40], s1tot[:, :], sc)

        nc.sync.dma_start(out[:, :], O[:, :])

    nc.finalize()
    return nc


def _get_nc():
    if "nc" not in _CACHE:
        _CACHE["nc"] = _build()
    return _CACHE["nc"]


def _host_inputs():
    iota = (np.arange(128, dtype=np.float32)[None, :]
            + 128.0 * np.tile(np.arange(4, dtype=np.float32), 8)[:, None])
    ident = np.eye(128, dtype=np.float32)
    ones = np.ones((128, 1), dtype=np.float32)
    return iota, ident, ones


def _run(seg_mask, trace=False):
    from concourse.bass_utils import run_bass_kernel_spmd

    x = np.ascontiguousarray(np.asarray(seg_mask, dtype=np.float32))
    assert x.shape == (B, C, H, W)
    iota, ident, ones = _host_inputs()
    in_maps = [
        {"x": x[SPC * c:SPC * (c + 1)], "iota": iota, "ident": ident,
         "ones": ones}
        for c in range(NCORES)
    ]
    nc = _get_nc()
    res = run_bass_kernel_spmd(nc, in_maps, core_ids=list(range(NCORES)),
                               trace=trace)
    outs = []
    for c in range(NCORES):
        o = np.asarray(res.results[c]["out"]).reshape(5, SPC).T
        outs.append(o)
    full = np.concatenate(outs, axis=0).astype(np.float32)
    return full, res


def kernel(segmentation_mask):
    full, _ = _run(segmentation_mask, trace=False)
    return full
